# revision 9
# baseline (speedup 1.0000x reference)
"""MinLSTM cell for Trainium2 (Bass/Tile), data-parallel over batch on 8 cores.

Per core (one batch row):
  - x staged as [128, KD, T] bf16 (host pre-transposed) so each time-chunk
    loads with ONE DMA; weights as kd-pair tiles [128, 2, H] (3 DMAs per
    projection) so the first f-matmuls start as soon as pair 0 lands.
  - three projections W^T.T @ xT -> [H,T] with bf16 matmuls (1 cyc/row,
    fast-weight-load) accumulating K=768 in PSUM.
  - division-free gates: with Ef = e^{-zf}, Ei = e^{-zi} (ScalarE Exp from
    PSUM, bias fused), a = (1+Ei)/(2+Ef+Ei), u = 1-a, and 1/s2 via
    Exp(-Ln(ss+2)) — Exp/Ln from one LUT table.
  - engines: ACT {ef, ei, ln, rt}, Pool {ss, ut}, DVE {at, bt, scan};
    stage2 lags stage1 by one j so ACT never idles on Pool's ss.
  - recurrence h_t = a_t*h_{t-1} + b_t as one VectorE tensor_tensor_scan
    per [128,w] tile, chained across chunks via the previous chunk's last
    column. Last 512 steps split into two 256-wide chunks (p-major) so only
    j=5's bt+scan+store trail the final matmul.
  - output hT [H,T] bf16; host transposes/upcasts to [T,H] fp32.
"""

import sys

if "/opt/trn_rl_repo" not in sys.path:
    sys.path.insert(0, "/opt/trn_rl_repo")

import numpy as np

B, T, D, H = 8, 4096, 768, 768
TC = 512                    # steady-state time-chunk (one PSUM bank of fp32)
KD = D // 128               # 6 contraction tiles
MH = H // 128               # 6 hidden tiles
CHUNKS = [(c * TC, TC) for c in range(7)] + [(3584, 256), (3840, 256)]

_state = {}


def _build():
    import concourse.mybir as mybir
    import concourse.tile as tile
    from concourse import bacc

    f32, bf16 = mybir.dt.float32, mybir.dt.bfloat16
    A = mybir.AluOpType
    Act = mybir.ActivationFunctionType

    nc = bacc.Bacc("TRN2", target_bir_lowering=False, debug=False, num_devices=B)

    xP = nc.dram_tensor("xP", [128, KD, T], bf16, kind="ExternalInput")
    w_d = {p: nc.dram_tensor(f"w{p}", [128, KD, H], bf16, kind="ExternalInput") for p in "fih"}
    bc_d = nc.dram_tensor("bcat", [128, 4 * MH], f32, kind="ExternalInput")
    hT = nc.dram_tensor("hT", [H, T], bf16, kind="ExternalOutput")

    with tile.TileContext(nc) as tc:
        with (
            tc.tile_pool(name="wpool", bufs=1) as wpool,
            tc.tile_pool(name="cpool", bufs=1) as cpool,
            tc.tile_pool(name="xpool", bufs=3) as xpool,
            tc.tile_pool(name="pspool", bufs=8, space="PSUM") as pspool,
            tc.tile_pool(name="wk", bufs=6) as wk,
            tc.tile_pool(name="hpool", bufs=3) as hpool,
        ):
            # Chunk-0 x first (one DMA), then weights as kd-pairs on three
            # queues in parallel: f first (chunk 0 streams f-groups first).
            x0 = xpool.tile([128, KD, TC], bf16, tag="x", name="x0")
            nc.sync.dma_start(x0[:], xP[:, :, 0:TC])
            w_q = {"f": nc.gpsimd, "i": nc.scalar, "h": nc.sync}
            w_sb = {p: [] for p in "fih"}
            for p in "fih":
                for kp in range(KD // 2):
                    t = wpool.tile([128, 2, H], bf16, tag=f"w{p}{kp}", name=f"w{p}{kp}")
                    w_q[p].dma_start(t[:], w_d[p][:, 2 * kp:2 * kp + 2, :])
                    w_sb[p].append(t)
            bcat = cpool.tile([128, 4 * MH], f32, tag="bc")
            nc.gpsimd.dma_start(bcat[:], bc_d[:])
            b_sb = {"f": bcat[:, 0:MH], "i": bcat[:, MH:2 * MH], "h": bcat[:, 2 * MH:3 * MH]}
            h0_sb = bcat[:, 3 * MH:4 * MH]
            two_sb = cpool.tile([128, 1], f32, tag="two")
            nc.gpsimd.memset(two_sb[:], 2.0)

            prev_h = [None] * MH
            prev_w = TC
            for ci, (tc0, w) in enumerate(CHUNKS):
                if ci == 0:
                    xall = x0
                else:
                    xall = xpool.tile([128, KD, TC], bf16, tag="x", name=f"x{ci}")
                    nc.sync.dma_start(xall[:, :, 0:w], xP[:, :, tc0:tc0 + w])

                def emit_group(p, j, ps):
                    pt = pspool.tile([128, TC], f32, tag="ps", name=f"ps{ci}_{j}_{p}")
                    for kd in range(KD):
                        nc.tensor.matmul(
                            pt[:, 0:w],
                            w_sb[p][kd // 2][:, kd % 2:kd % 2 + 1, j * 128:(j + 1) * 128],
                            xall[:, kd:kd + 1, 0:w],
                            start=(kd == 0),
                            stop=(kd == KD - 1),
                        )
                    ps[p] = pt

                PS = [dict() for _ in range(MH)]
                S1 = [None] * MH

                # Division-free gates via Exp/Ln (single ACT table): with
                # Ef = e^{-zf}, Ei = e^{-zi}:  f/(f+i) = (1+Ei)/(2+Ef+Ei)
                # and 1/s2 = Exp(-Ln(ss+2)) with the +2 in Ln's bias AP;
                # u = 1-a replaces the (1+Ef)*rt product. stage1 depends only
                # on the f/i projections; stage2 is emitted one j late so ACT
                # never stalls waiting for Pool's ss.
                def stage1(j):
                    ps = PS[j]
                    ef = wk.tile([128, TC], bf16, tag="ef", name=f"ef{ci}_{j}")
                    nc.scalar.activation(ef[:, 0:w], ps["f"][:, 0:w], Act.Exp, bias=b_sb["f"][:, j:j + 1], scale=-1.0)
                    ei = wk.tile([128, TC], bf16, tag="ei", name=f"ei{ci}_{j}")
                    nc.scalar.activation(ei[:, 0:w], ps["i"][:, 0:w], Act.Exp, bias=b_sb["i"][:, j:j + 1], scale=-1.0)
                    ss = wk.tile([128, TC], bf16, tag="s2", name=f"ss{ci}_{j}")
                    nc.gpsimd.tensor_tensor(ss[:, 0:w], ef[:, 0:w], ei[:, 0:w], A.add)
                    S1[j] = (ef, ei, ss)

                def stage2(j):
                    ps = PS[j]
                    ef, ei, ss = S1[j]
                    ln2 = wk.tile([128, TC], f32, tag="ln2", name=f"ln{ci}_{j}")
                    nc.scalar.activation(ln2[:, 0:w], ss[:, 0:w], Act.Ln, bias=two_sb[:, 0:1], scale=1.0)
                    rt = wk.tile([128, TC], bf16, tag="rt", name=f"rt{ci}_{j}")
                    nc.scalar.activation(rt[:, 0:w], ln2[:, 0:w], Act.Exp, bias=0.0, scale=-1.0)
                    at = wk.tile([128, TC], bf16, tag="a", name=f"at{ci}_{j}")
                    nc.vector.scalar_tensor_tensor(at[:, 0:w], ei[:, 0:w], 1.0, rt[:, 0:w], A.add, A.mult)
                    ut = wk.tile([128, TC], bf16, tag="u", name=f"ut{ci}_{j}")
                    nc.gpsimd.tensor_scalar(ut[:, 0:w], at[:, 0:w], scalar1=-1.0, scalar2=1.0, op0=A.mult, op1=A.add)
                    bt = wk.tile([128, TC], bf16, tag="b", name=f"bt{ci}_{j}")
                    nc.vector.scalar_tensor_tensor(bt[:, 0:w], ps["h"][:, 0:w], b_sb["h"][:, j:j + 1], ut[:, 0:w], A.add, A.mult)
                    hh = hpool.tile([128, TC], bf16, tag=f"h{j}", name=f"hh{ci}_{j}")
                    init = h0_sb[:, j:j + 1] if ci == 0 else prev_h[j][:, prev_w - 1:prev_w]
                    nc.vector.tensor_tensor_scan(hh[:, 0:w], at[:, 0:w], bt[:, 0:w], init, op0=A.mult, op1=A.add)
                    prev_h[j] = hh
                    nc.sync.dma_start(hT[j * 128:(j + 1) * 128, tc0:tc0 + w], hh[:, 0:w])

                if ci == 0 or ci == len(CHUNKS) - 1:
                    # p-major: ci=0 streams all f-groups while wi/wh DMAs are
                    # in flight; the last chunk runs every gate chain during
                    # the h-group matmuls so only j=5's bt+scan+store trail
                    # the final matmul.
                    for j in range(MH):
                        emit_group("f", j, PS[j])
                    for j in range(MH):
                        emit_group("i", j, PS[j])
                        stage1(j)
                    for j in range(MH):
                        emit_group("h", j, PS[j])
                        stage2(j)
                else:
                    for j in range(MH):
                        emit_group("f", j, PS[j])
                        emit_group("i", j, PS[j])
                        stage1(j)
                        emit_group("h", j, PS[j])
                        if j > 0:
                            stage2(j - 1)
                    stage2(MH - 1)
                prev_w = w

    # All our ACT funcs (Exp, Ln, Identity, Copy) live in the single table
    # "natural_log_exp_and_others", but the table-load pass picks the FIRST
    # table containing each func, thrashing Exp->exp_and_others /
    # Ln->natural_log (96 swaps x 1.3us). Empty out every other table (names
    # and positions preserved, so emitted runtime table ids stay valid) so
    # first-match lands on the one shared table and a single load is emitted.
    import concourse.bacc as bacc_mod

    orig_tables = bacc_mod.get_activation_tables

    def _single_table(arch):
        tabs = orig_tables(arch)
        keep = "natural_log_exp_and_others"
        return {k: (v if k == keep else set()) for k, v in tabs.items()}

    bacc_mod.get_activation_tables = _single_table
    try:
        nc.compile()
    finally:
        bacc_mod.get_activation_tables = orig_tables
    return nc


def _get_nc():
    if "nc" not in _state:
        _state["nc"] = _build()
    return _state["nc"]


def _prep_inputs(x, h0, f_w, f_b, i_w, i_b, h_w, h_b):
    import ml_dtypes

    bf16 = ml_dtypes.bfloat16
    x = np.asarray(x, dtype=np.float32)
    h0 = np.asarray(h0, dtype=np.float32)
    # [B, D, T] -> pair-major [B, 128, KD, T] so each chunk is one DMA
    xT = x.transpose(0, 2, 1).reshape(B, KD, 128, T).transpose(0, 2, 1, 3)
    xT = np.ascontiguousarray(xT.astype(bf16))
    shared = {}
    biases = []
    for p, wgt, bias, sgn in (("f", f_w, f_b, -1.0), ("i", i_w, i_b, -1.0), ("h", h_w, h_b, 1.0)):
        wgt = np.asarray(wgt, dtype=np.float32)
        # f/i biases negated: kernel computes Exp(-pre + bias_ap), needs bias_ap = -b
        biases.append((sgn * np.asarray(bias, dtype=np.float32)).reshape(MH, 128).T)
        wP = wgt.T.reshape(KD, 128, H).transpose(1, 0, 2)   # [128, KD, H]
        shared[f"w{p}"] = np.ascontiguousarray(wP.astype(bf16))
    in_maps = []
    for b in range(B):
        m = dict(shared)
        m["xP"] = xT[b]
        h0c = h0[b, 0].reshape(MH, 128).T
        m["bcat"] = np.ascontiguousarray(np.concatenate(biases + [h0c], axis=1))
        in_maps.append(m)
    return in_maps


def kernel(x, h0, f_w, f_b, i_w, i_b, h_w, h_b, _trace=False):
    from concourse.bass_utils import run_bass_kernel_spmd

    nc = _get_nc()
    in_maps = _prep_inputs(x, h0, f_w, f_b, i_w, i_b, h_w, h_b)
    res = run_bass_kernel_spmd(nc, in_maps, core_ids=list(range(B)), trace=_trace)
    out = np.empty((B, T, H), dtype=np.float32)
    for b in range(B):
        out[b] = res.results[b]["hT"].T.astype(np.float32)
    if _trace:
        _state["last_results"] = res
    return out
